# revision 19
# baseline (speedup 1.0000x reference)
"""CondConv2d (MoE-routed 3x3 conv) Trainium2 Bass kernel, v2.

Full-input contract: kernel(**inputs) takes the unsharded tensors and
returns the full [32, 192, 56, 56] output. Data-parallel across batch
over 8 NeuronCores (4 samples per core); each core mixes its own
per-sample weights on-device and runs its samples' convolutions.

v2 layout (transposed matmul orientation, bf16):
  out[p, o] = sum over (i, dy, dx) of xf[i, p + d(dy,dx)] * w[o, i, dy, dx]
with x stored flat 58-col-padded ([128, 3480] per tile, host-prepadded)
so every conv offset is a shifted 1D view: d(dy,dx) = (dy-1)*58+(dx-1).
Each matmul: lhsT = x-view [K<=128 chans, M=128 flat pixels], rhs =
mixed weights [K, N=192 outchans], accumulating in a [128, 192] f32
PSUM tile. bf16 runs at 1 cycle/row (fp32r would be 4x at N<256), so a
tile costs 14 matmuls x 192 rows. Contraction packing per tile:
  - head chans 0:128: 9 chunks, one per (dy, dx), K=128
  - tail chans 128:192 twice per tile: (A; B=A shifted one row) covers
    (dy0, dy1) pairs per dx -> 3 chunks K=128; (A; D=A shifted one col)
    covers (dy2,dx0)+(dy2,dx1) in one K=128 chunk; (A; ones) covers
    (dy2,dx2) with K=65 where partition 64 = ones row x rhs bias row,
    folding the routed bias into the accumulation for free.
  => 14 matmuls per 128-pixel tile, 26 tiles per sample.
Mixing (routing @ experts) is chained scalar_tensor_tensor, split
across DVE (head groups) and Pool (tail tiles) so neither exceeds the
PE's ~29us/sample; PSUM is drained by the scalar engine to bf16 and
host re-strides [3328, 192] -> [192, 56, 56] f32 per sample.
"""

import numpy as np

B, E = 32, 8
O, I = 192, 192
H, W = 56, 56
NCORES = 8
BPC = B // NCORES  # samples per core
FP = 3480  # flat padded x length per channel (58*58=3364 rounded up)
NT = 26  # 128-pixel tiles per sample (covers flat 59..3387)
P0 = 59  # first valid out position in padded-flat coords
NPS = 7  # PSUM tiles in flight (8 banks minus one for the rb matmul)

_CACHE = {}


def _build():
    import concourse.bass as bass  # noqa: F401
    from concourse import bacc, mybir, tile

    dt = mybir.dt
    f32 = dt.float32
    bf16 = dt.bfloat16
    MULT = mybir.AluOpType.mult
    ADD = mybir.AluOpType.add
    IDENT = mybir.ActivationFunctionType.Identity

    nc = bacc.Bacc(
        "TRN2",
        target_bir_lowering=False,
        debug=False,
        enable_asserts=False,
        num_devices=NCORES,
    )

    # x, host-prepadded flat bf16 per sample
    xh_d = nc.dram_tensor("xh", [BPC, 128, FP], bf16, kind="ExternalInput").ap()
    xtp_d = nc.dram_tensor("xtp", [BPC, 128, FP], bf16, kind="ExternalInput").ap()
    xtd_d = nc.dram_tensor("xtd", [BPC, 128, FP], bf16, kind="ExternalInput").ap()
    xt2_d = nc.dram_tensor("xt2", [BPC, 65, FP], bf16, kind="ExternalInput").ap()
    # weights: wht free = ((dy*E + e)*3 + dx)*O + o
    wht_d = nc.dram_tensor("wht", [128, 9 * E * O], bf16, kind="ExternalInput").ap()
    # fused tail consts, free = e*960 + c with c: 0:576 pair (A: dy0,
    # B: dy1), 576:768 colpair (A: dy2dx0, D: dy2dx1), 768:960 dy2dx2
    # on partitions 0:64 with bias[e] on partition 64, zeros above
    wtt_d = nc.dram_tensor("wtt", [128, E * 960], bf16, kind="ExternalInput").ap()
    rf_d = nc.dram_tensor("rf", [1, BPC * E], f32, kind="ExternalInput").ap()
    out_d = nc.dram_tensor("out", [BPC, NT * 128, O], bf16, kind="ExternalOutput").ap()

    # flat-view offset for conv tap (dy, dx)
    def dlt(dy, dx):
        return (dy - 1) * 58 + (dx - 1)

    with tile.TileContext(nc) as tc:
        with (
            tc.tile_pool(name="consts", bufs=1) as consts,
            tc.tile_pool(name="xp", bufs=3) as xp,
            tc.tile_pool(name="wm", bufs=3) as wm_pool,
            tc.tile_pool(name="stage", bufs=6) as stage_pool,
            tc.tile_pool(name="psum1", bufs=1, space="PSUM") as psum1,
            tc.tile_pool(name="cpsum", bufs=1, space="PSUM") as cpsum,
        ):
            # ---- routing broadcast to all partitions via K=1 ones matmul
            rf_sb = consts.tile([1, BPC * E], f32)
            nc.sync.dma_start(out=rf_sb, in_=rf_d)
            ones_sb = consts.tile([1, 128], f32)
            nc.vector.memset(ones_sb, 1.0)
            ps0 = psum1.tile([128, BPC * E], f32)
            nc.tensor.matmul(ps0, lhsT=ones_sb, rhs=rf_sb, start=True, stop=True)
            rb = consts.tile([128, BPC * E], f32)
            nc.vector.tensor_copy(rb, ps0)

            # DMA_ENGINES is one shared serial resource in the HW model, so
            # arrival order is what matters: weights first (mix chains start
            # chasing them), then x0 on the same ring; later x and all
            # output DMAs go via the SP ring.
            def emit_x_dma(b):
                # sample 0 is chunked so conv tiles chase the arriving
                # stream via subtile deps; later samples land whole while
                # the previous sample's conv runs
                nchunk = 2 if b == 0 else 1
                xhb = xp.tile([128, FP], bf16, tag="xh", name="xh")
                xtpb = xp.tile([128, FP], bf16, tag="xtp", name="xtp")
                xtdb = xp.tile([128, FP], bf16, tag="xtd", name="xtd")
                xt2b = xp.tile([65, FP], bf16, tag="xt2", name="xt2")
                hc = FP // nchunk
                for c in range(nchunk):
                    sl = slice(c * hc, (c + 1) * hc)
                    nc.sync.dma_start(out=xhb[:, sl], in_=xh_d[b, :, sl])
                    nc.sync.dma_start(out=xtpb[:, sl], in_=xtp_d[b, :, sl])
                    nc.sync.dma_start(out=xtdb[:, sl], in_=xtd_d[b, :, sl])
                    nc.sync.dma_start(out=xt2b[:, sl], in_=xt2_d[b, :, sl])
                return xhb, xtpb, xtdb, xt2b

            # ---- resident expert weights, one DMA per group (per-DMA ring
            # overhead is ~254ns, so batch); ordered dy0, dy1, wtt, dy2 to
            # match the DVE head-chain / ACT+Pool tail-mix consumption order
            wht = consts.tile([128, 3, E, 3 * O], bf16)  # [i, dy, e, (dx, o)]
            wtt = consts.tile([128, E, 960], bf16)  # fused tail consts
            GSZ = E * 3 * O
            nc.sync.dma_start(out=wht[:, 0, :, :], in_=wht_d[:, 0:GSZ])
            nc.sync.dma_start(out=wht[:, 1, :, :], in_=wht_d[:, GSZ:2 * GSZ])
            nc.sync.dma_start(out=wtt, in_=wtt_d)
            nc.sync.dma_start(out=wht[:, 2, :, :], in_=wht_d[:, 2 * GSZ:3 * GSZ])

            xt = {0: emit_x_dma(0)}

            def emit_mix(b):
                # head: chained scalar_tensor_tensor MAC on DVE (sample 0
                # runs per-dy chains so mixing pipelines behind the wht
                # DMA; later samples use one 3D op per expert); tail: ACT
                # does tmp = W_e * r (per-partition scale), Pool
                # accumulates wmt += tmp (TensorScalarPtr is not legal on
                # Pool, tensor_tensor is).
                wmh = wm_pool.tile([128, 3, 3 * O], bf16, tag="wmh", name="wmh")
                wmt = wm_pool.tile([128, 960], bf16, tag="wmt", name="wmt")
                if b == 0:
                    # per-dy chains chase the wht DMA stream; tail on
                    # ACT (scaled copy) + Pool (tensor_tensor add) so it
                    # runs concurrently with the DVE head chains
                    for dy in range(3):
                        for e in range(E):
                            rc = rb[:, b * E + e:b * E + e + 1]
                            src = wht[:, dy, e, :]
                            dst = wmh[:, dy, :]
                            if e == 0:
                                nc.vector.tensor_scalar_mul(dst, src, rc)
                            else:
                                nc.vector.scalar_tensor_tensor(
                                    dst, src, rc, dst, op0=MULT, op1=ADD)
                    for e in range(E):
                        rc = rb[:, b * E + e:b * E + e + 1]
                        if e == 0:
                            nc.scalar.activation(wmt, wtt[:, 0, :], IDENT,
                                                 scale=rc)
                        else:
                            tmp = wm_pool.tile([128, 960], bf16, tag="tmt",
                                               name="tmt")
                            nc.scalar.activation(tmp, wtt[:, e, :], IDENT,
                                                 scale=rc)
                            nc.gpsimd.tensor_tensor(wmt, wmt, tmp, op=ADD)
                else:
                    # later samples: everything on DVE, decoupled from the
                    # in-order ACT drain stream (24.8us < 29.1us conv)
                    for e in range(E):
                        rc = rb[:, b * E + e:b * E + e + 1]
                        src = wht[:, :, e, :]
                        if e == 0:
                            nc.vector.tensor_scalar_mul(wmh, src, rc)
                        else:
                            nc.vector.scalar_tensor_tensor(
                                wmh, src, rc, wmh, op0=MULT, op1=ADD)
                    for e in range(E):
                        rc = rb[:, b * E + e:b * E + e + 1]
                        if e == 0:
                            nc.vector.tensor_scalar_mul(wmt, wtt[:, 0, :], rc)
                        else:
                            nc.vector.scalar_tensor_tensor(
                                wmt, wtt[:, e, :], rc, wmt, op0=MULT, op1=ADD)
                return wmh, wmt

            wm = {0: emit_mix(0)}

            for b in range(BPC):
                if b + 1 < BPC:
                    wm[b + 1] = emit_mix(b + 1)
                xhb, xtpb, xtdb, xt2b = xt.pop(b)
                wmh, wmt = wm.pop(b)

                for t in range(NT):
                    if t == 3 and b + 1 < BPC:
                        # deferred so x[b+1]'s transfers queue behind conv
                        # b's first drains instead of competing with the
                        # startup weight/x0 stream on DMA_ENGINES
                        xt[b + 1] = emit_x_dma(b + 1)
                    ps = cpsum.tile([128, O], f32, tag=f"cps{t % NPS}",
                                    name=f"cps{t % NPS}")
                    s0 = P0 + t * 128
                    # last-ready inputs FIRST (k65 needs the Pool-mixed wmt
                    # and the final x chunk): the PE p-state ramp resets on
                    # any stall, so gate the stream on the slowest dep and
                    # then run gapless at full clock
                    s = s0 + dlt(2, 2)
                    nc.tensor.matmul(ps, lhsT=xt2b[0:65, s:s + 128],
                                     rhs=wmt[0:65, 768:960],
                                     start=True, stop=False)
                    # tail (dy2, dx0)+(dy2, dx1) via col-shifted pair
                    s = s0 + dlt(2, 0)
                    nc.tensor.matmul(ps, lhsT=xtdb[:, s:s + 128],
                                     rhs=wmt[:, 576:768],
                                     start=False, stop=False)
                    # tail (dy0, dy1) pairs per dx
                    for dx in range(3):
                        s = s0 + dlt(0, dx)
                        nc.tensor.matmul(
                            ps, lhsT=xtpb[:, s:s + 128],
                            rhs=wmt[:, dx * O:(dx + 1) * O],
                            start=False, stop=False)
                    # head chunks, dy2 (latest-mixed) first
                    for ci, (dy, dx) in enumerate(
                            (dy, dx) for dy in (2, 1, 0) for dx in range(3)):
                        s = s0 + dlt(dy, dx)
                        nc.tensor.matmul(
                            ps, lhsT=xhb[:, s:s + 128],
                            rhs=wmh[:, dy, dx * O:(dx + 1) * O],
                            start=False, stop=(ci == 8))

                    st = stage_pool.tile([128, O], bf16, tag="st", name="st")
                    nc.scalar.activation(st, ps, IDENT)
                    nc.scalar.dma_start(out=out_d[b, t * 128:(t + 1) * 128, :],
                                        in_=st)

    nc.compile()
    return nc


def _prep_inputs(x, routing_weights, weight, bias):
    import ml_dtypes

    bf = ml_dtypes.bfloat16
    x = np.asarray(x, np.float32)
    routing = np.asarray(routing_weights, np.float32)
    weight = np.asarray(weight, np.float32)
    bias = np.asarray(bias, np.float32)

    W5 = weight.reshape(E, O, I, 3, 3)
    # head: [i, dy, e, dx, o]
    wht_h = np.ascontiguousarray(
        W5[:, :, :128].transpose(2, 3, 0, 4, 1)).reshape(128, 9 * E * O)
    # fused tail consts [128, e, 960]:
    #   cols 0:576  pair (partitions A = tail dy0, B = tail dy1), (dx, o)
    #   cols 576:768 colpair (A = dy2dx0, D = dy2dx1)
    #   cols 768:960 dy2dx2 on partitions 0:64, bias on partition 64
    wtt_h = np.zeros((128, E, 960), np.float32)
    t0 = W5[:, :, 128:, 0, :].transpose(2, 0, 3, 1).reshape(64, E, 576)
    t1 = W5[:, :, 128:, 1, :].transpose(2, 0, 3, 1).reshape(64, E, 576)
    wtt_h[:64, :, 0:576] = t0
    wtt_h[64:, :, 0:576] = t1
    wtt_h[:64, :, 576:768] = W5[:, :, 128:, 2, 0].transpose(2, 0, 1)
    wtt_h[64:, :, 576:768] = W5[:, :, 128:, 2, 1].transpose(2, 0, 1)
    wtt_h[:64, :, 768:960] = W5[:, :, 128:, 2, 2].transpose(2, 0, 1)
    wtt_h[64, :, 768:960] = bias

    wht_b = wht_h.astype(bf)
    wtt_b = wtt_h.reshape(128, E * 960).astype(bf)

    # flat 58-padded x (+ room for view overhang), bf16
    xf = np.zeros((B, I, 60, 58), np.float32)
    xf[:, :, 1:57, 1:57] = x
    xf = xf.reshape(B, I, FP).astype(bf)
    xh_h = np.ascontiguousarray(xf[:, :128])  # [B, 128, FP]
    A = xf[:, 128:]  # [B, 64, FP] tail, unshifted
    sh58 = np.zeros_like(A)
    sh58[:, :, :FP - 58] = A[:, :, 58:]
    sh1 = np.zeros_like(A)
    sh1[:, :, :FP - 1] = A[:, :, 1:]
    xtp_h = np.ascontiguousarray(np.concatenate([A, sh58], axis=1))
    xtd_h = np.ascontiguousarray(np.concatenate([A, sh1], axis=1))
    ones_row = np.ones((B, 1, FP), np.float32).astype(bf)
    xt2_h = np.ascontiguousarray(np.concatenate([A, ones_row], axis=1))

    in_maps = []
    for c in range(NCORES):
        sl = slice(c * BPC, (c + 1) * BPC)
        in_maps.append({
            "xh": xh_h[sl],
            "xtp": xtp_h[sl],
            "xtd": xtd_h[sl],
            "xt2": xt2_h[sl],
            "wht": wht_b,
            "wtt": wtt_b,
            "rf": np.ascontiguousarray(routing[sl].reshape(1, BPC * E)),
        })
    return in_maps


def _run(in_maps, **kwargs):
    from concourse import bass_utils
    if "nc" not in _CACHE:
        _CACHE["nc"] = _build()
    return bass_utils.run_bass_kernel_spmd(
        _CACHE["nc"], in_maps, core_ids=list(range(NCORES)), **kwargs)


def kernel(x, routing_weights, weight, bias):
    in_maps = _prep_inputs(x, routing_weights, weight, bias)
    res = _run(in_maps)
    out = np.empty((B, O, H, W), np.float32)
    for c in range(NCORES):
        arr = np.asarray(res.results[c]["out"]).astype(np.float32)
        # rows j of arr map to padded-flat position P0 + j; out pixel
        # (r, cc) lives at j = 58*r + cc
        v = arr[:, :3248].reshape(BPC, 56, 58, O)[:, :, :56]
        out[c * BPC:(c + 1) * BPC] = v.transpose(0, 3, 1, 2)
    return out
